# revision 25
# baseline (speedup 1.0000x reference)
"""Canny edge detector on 8 Trainium2 NeuronCores.

Strategy (v2):
 - Stage F (flat): per channel, the vertical 5-tap gaussian vb = G_v(img)
   runs on the TENSOR engine as a banded fp32 matmul (exact to ~2e-7),
   in a flat rows-on-partitions layout (3 row-chains per core), written
   to an internal DRAM scratch `vbd`.
 - Stage B (blocked): identical to the v1 kernel but starts from vb
   (convs commute: G_h then A3_v/D3_v etc.), i.e. the 4 vertical-gaussian
   DVE ops per channel are gone.  Columns-on-partitions layout: partition
   p owns output cols [16p,16p+16) with a 26-wide halo window, all
   stencils are free-dim AP offsets.
 - Math (fp32, faithful to the reference):
     vb  = 5-tap vertical gaussian on img          (PE, stage F)
     bh  = 5-tap horizontal gaussian on vb         (DVE)
     t1  = vertical [1,2,1] of bh;  t2 = vertical [1,0,-1] of bh
     gx  = horizontal [1,0,-1] of t1;  gy = horizontal [1,2,1] of t2
     m_c = sqrt(gx^2+gy^2); g = sum_c m_c; sgx = sum_c gx; sgy = sum_c gy
     axis classification via |sgy| vs tan(22.5/67.5)*|sgx| and sign(sgx*sgy)
     nms+thresholds fused: cc = max of the 2 neighbors along the axis;
       hp = g > max(cc, high);  lm = g > max(cc, nextbefore(low))
     hysteresis: out = lm & max3x3(hp)   (binary planes in fp16)
"""

import numpy as np

_COMPILED = {}

H = 2048
W = 2048
HALO = 5
ROWS_PER_CORE = H // 8            # 256
SHARD_ROWS = ROWS_PER_CORE + 2 * HALO   # 266
PADW = W + 2 * HALO               # 2058
VB_ROWS = ROWS_PER_CORE + 6       # 262 (vb needs +-3 rows)
N_CHUNK = 128                     # output rows per chunk
CHUNKS = [(r, r + N_CHUNK) for r in range(0, ROWS_PER_CORE, N_CHUNK)]

# flat-stage chains: (img slab row start, img rows, out valid local range,
#                     vbd row start)   vb slab row v+2 == vbd row v
# chain A: img [0,128)   -> vb slab [2,126)   -> vbd [0,124)
# chain B: img [122,250) -> vb slab [124,248) -> vbd [122,244), write [124,244)
# chain C: img [244,266) -> vb slab [246,264) -> vbd [244,262)
SEGS = [(0, 512), (512, 512), (1024, 512), (1536, 512), (2048, 10)]


def _gauss5():
    return np.exp(-0.5 * (np.arange(5) - 2.0) ** 2).astype(np.float32)


def _wg_host():
    g5 = _gauss5()
    Wg = np.zeros((128, 124), np.float32)
    for i in range(124):
        for k in range(5):
            Wg[i + k, i] = g5[k]
    return Wg


def _build(low, high):
    import concourse.bass as bass
    import concourse.bacc as bacc
    import concourse.mybir as mybir
    from concourse.tile import TileContext

    f32 = mybir.dt.float32
    Alu = mybir.AluOpType
    Act = mybir.ActivationFunctionType

    g5 = _gauss5()
    ga = float(g5[0])
    gb = float(g5[1])
    t1c = float(np.float32(np.tan(np.deg2rad(np.float64(22.5)))))
    t2c = float(np.float32(np.tan(np.deg2rad(np.float64(67.5)))))

    nc = bacc.Bacc()
    x = nc.dram_tensor("x", [3, SHARD_ROWS, PADW], f32, kind="ExternalInput")
    wg = nc.dram_tensor("wg", [128, 124], f32, kind="ExternalInput")
    vbd = nc.dram_tensor("vbd", [3, VB_ROWS, PADW], f32, kind="Internal")
    f16o = mybir.dt.float16
    out = nc.dram_tensor("out", [ROWS_PER_CORE, W], f16o, kind="ExternalOutput")

    with TileContext(nc) as tc:
        with tc.tile_pool(name="io", bufs=2) as iop, tc.tile_pool(
            name="pl", bufs=1
        ) as pool, tc.tile_pool(name="fl", bufs=2) as flp, tc.tile_pool(
            name="ps", bufs=4, space="PSUM"
        ) as psp:
            # ---- Stage F: vb = G_v(img) on the PE, per channel/chain ----
            wgs = pool.tile([128, 124], f32, tag="wg")
            nc.sync.dma_start(out=wgs[:], in_=wg[:, :])

            # warm up the PE HAM clock gate with a dense dummy burst
            wps = psp.tile([124, 124], f32, tag="warm")
            for _ in range(10):
                nc.tensor.matmul(wps[:], lhsT=wgs[0:128, 0:124],
                                 rhs=wgs[0:128, 0:124], start=True, stop=True)

            # (img_row0, n_img_rows, lhsT cols (out rows), vb local base,
            #  vbd row0, n out rows written, local out row offset)
            chains = [
                (0, 128, 124, 0, 124, 0),
                (122, 128, 122, 124, 120, 2),
                (244, 22, 18, 244, 18, 0),
            ]

            def emit_flat_ch(c, chain_sel):
                for (i0, nr, m, v0, nw, oo) in chain_sel:
                    imgf = flp.tile([nr, PADW], f32, tag="imgf")
                    src = bass.AP(
                        x, c * SHARD_ROWS * PADW + i0 * PADW,
                        [[PADW, nr], [1, PADW]],
                    )
                    nc.sync.dma_start(out=imgf[:], in_=src)
                    vbf = flp.tile([m, PADW], f32, tag="vbf")
                    for (s0, sl) in SEGS:
                        ps = psp.tile([m, sl], f32, tag="ps")
                        nc.tensor.matmul(
                            ps[:], lhsT=wgs[0:nr, 0:m],
                            rhs=imgf[:, s0:s0 + sl],
                            start=True, stop=True,
                        )
                        nc.scalar.copy(vbf[:, s0:s0 + sl], ps[:])
                    dst = bass.AP(
                        vbd, c * VB_ROWS * PADW + v0 * PADW,
                        [[PADW, nw], [1, PADW]],
                    )
                    nc.sync.dma_start(out=dst, in_=vbf[oo:oo + nw, :])

            # ---- Stage B: blocked NMS pipeline from vbd ----
            # partition p = (h, b): row-half h (64 rows) x col-block b
            # (32 cols, 42-wide halo window)
            def blocked_planes():
                gpl = pool.tile([128, 68, 36], f32, tag="g")
                sgx = pool.tile([128, 68, 36], f32, tag="sgx")
                sgy = pool.tile([128, 68, 36], f32, tag="sgy")
                return gpl, sgx, sgy

            def emit_vbt_load(r0, c):
                vbt = iop.tile([128, 70, 42], f32, tag="img")
                for h in (0, 1):
                    src = bass.AP(
                        vbd, c * VB_ROWS * PADW + (r0 + 64 * h) * PADW,
                        [[32, 64], [PADW, 70], [1, 42]],
                    )
                    nc.sync.dma_start(out=vbt[64 * h:64 * h + 64, :, :], in_=src)
                return vbt

            def emit_blocked_ch(planes, r0, c, vbt=None):
                gpl, sgx, sgy = planes
                if True:
                    if vbt is None:
                        vbt = emit_vbt_load(r0, c)

                    s1 = pool.tile([128, 70, 38], f32, tag="tA")
                    s2 = pool.tile([128, 70, 38], f32, tag="tB")
                    bh1 = pool.tile([128, 70, 38], f32, tag="tC")
                    bh = pool.tile([128, 70, 38], f32, tag="tD")
                    # horizontal 5-tap gaussian [ga, gb, 1, gb, ga]
                    nc.vector.tensor_tensor(s1[:], vbt[:, :, 1:39], vbt[:, :, 3:41], Alu.add)
                    nc.vector.tensor_tensor(s2[:], vbt[:, :, 0:38], vbt[:, :, 4:42], Alu.add)
                    nc.vector.scalar_tensor_tensor(
                        bh1[:], s1[:], gb, vbt[:, :, 2:40], Alu.mult, Alu.add)
                    nc.vector.scalar_tensor_tensor(
                        bh[:], s2[:], ga, bh1[:], Alu.mult, Alu.add)

                    u = pool.tile([128, 68, 38], f32, tag="tA")
                    t1 = pool.tile([128, 68, 38], f32, tag="tB")
                    t2 = pool.tile([128, 68, 38], f32, tag="tC")
                    # vertical sobel components
                    nc.vector.tensor_tensor(u[:], bh[:, 0:68, :], bh[:, 2:70, :], Alu.add)
                    nc.vector.scalar_tensor_tensor(
                        t1[:], bh[:, 1:69, :], 2.0, u[:], Alu.mult, Alu.add)
                    nc.vector.tensor_tensor(t2[:], bh[:, 0:68, :], bh[:, 2:70, :], Alu.subtract)

                    gx = sgx if c == 0 else pool.tile([128, 68, 36], f32, tag="tD")
                    gy = sgy if c == 0 else pool.tile([128, 68, 36], f32, tag="tE")
                    w2 = pool.tile([128, 68, 36], f32, tag="tF")
                    # horizontal sobel components
                    nc.vector.tensor_tensor(gx[:], t1[:, :, 0:36], t1[:, :, 2:38], Alu.subtract)
                    nc.vector.tensor_tensor(w2[:], t2[:, :, 0:36], t2[:, :, 2:38], Alu.add)
                    nc.vector.scalar_tensor_tensor(
                        gy[:], t2[:, :, 1:37], 2.0, w2[:], Alu.mult, Alu.add)

                    q1 = pool.tile([128, 68, 36], f32, tag="tA")
                    q2 = pool.tile([128, 68, 36], f32, tag="tB")
                    r2 = pool.tile([128, 68, 36], f32, tag="tC")
                    m = gpl if c == 0 else pool.tile([128, 68, 36], f32, tag="tF")
                    nc.scalar.activation(q1[:], gx[:], Act.Square)
                    nc.scalar.activation(q2[:], gy[:], Act.Square)
                    nc.vector.tensor_tensor(r2[:], q1[:], q2[:], Alu.add)
                    nc.scalar.activation(m[:], r2[:], Act.Sqrt)

                    if c > 0:
                        nc.vector.tensor_tensor(gpl[:], gpl[:], m[:], Alu.add)
                        nc.vector.tensor_tensor(sgx[:], sgx[:], gx[:], Alu.add)
                        nc.vector.tensor_tensor(sgy[:], sgy[:], gy[:], Alu.add)

            def emit_blocked_nms(planes, r0):
                gpl, sgx, sgy = planes
                # ---- NMS ----
                u8 = mybir.dt.uint8
                rr = pool.tile([128, 66, 34], f32, tag="cand")
                ss = pool.tile([128, 66, 34], f32, tag="cand2")
                m0 = pool.tile([128, 66, 34], u8, tag="mk0")
                m2 = pool.tile([128, 66, 34], u8, tag="mk1")
                d = pool.tile([128, 66, 34], f32, tag="tE")
                dpos = pool.tile([128, 66, 34], u8, tag="mk2")
                nc.scalar.activation(rr[:], sgy[:, 1:67, 1:35], Act.Abs)
                nc.scalar.activation(ss[:], sgx[:, 1:67, 1:35], Act.Abs)
                nc.vector.scalar_tensor_tensor(m0[:], ss[:], t1c, rr[:], Alu.mult, Alu.is_ge)
                nc.vector.scalar_tensor_tensor(m2[:], ss[:], t2c, rr[:], Alu.mult, Alu.is_le)
                nc.vector.tensor_tensor(
                    d[:], sgx[:, 1:67, 1:35], sgy[:, 1:67, 1:35], Alu.mult)
                nc.vector.tensor_scalar(dpos[:], d[:], 0.0, None, Alu.is_ge)

                cand = pool.tile([128, 66, 34], f32, tag="cand")
                cc = pool.tile([128, 66, 34], f32, tag="cc")
                # base: c3 = max(SW, NE); overwrite with c1/c2/c0 by priority
                nc.vector.tensor_tensor(
                    cand[:], gpl[:, 2:68, 2:36], gpl[:, 0:66, 0:34], Alu.max)  # c1 SE/NW
                nc.vector.tensor_tensor(
                    cc[:], gpl[:, 2:68, 0:34], gpl[:, 0:66, 2:36], Alu.max)    # c3 SW/NE
                nc.vector.copy_predicated(cc[:], dpos[:], cand[:])
                cand2 = pool.tile([128, 66, 34], f32, tag="cand2")
                nc.vector.tensor_tensor(
                    cand2[:], gpl[:, 2:68, 1:35], gpl[:, 0:66, 1:35], Alu.max)  # c2 S/N
                nc.vector.copy_predicated(cc[:], m2[:], cand2[:])
                f16 = mybir.dt.float16
                cand3 = pool.tile([128, 66, 34], f32, tag="cand")
                nc.vector.tensor_tensor(
                    cand3[:], gpl[:, 1:67, 2:36], gpl[:, 1:67, 0:34], Alu.max)  # c0 E/W
                nc.vector.copy_predicated(cc[:], m0[:], cand3[:])

                hp = pool.tile([128, 66, 34], f16, tag="tF")
                lm = pool.tile([128, 64, 32], f16, tag="cand")
                lowx = float(np.nextafter(np.float32(low), np.float32(0.0)))
                nc.vector.scalar_tensor_tensor(
                    hp[:], cc[:], high, gpl[:, 1:67, 1:35], Alu.max, Alu.is_lt)
                nc.vector.scalar_tensor_tensor(
                    lm[:], cc[:, 1:65, 1:33], lowx, gpl[:, 2:66, 2:34],
                    Alu.max, Alu.is_lt)

                rm1 = pool.tile([128, 66, 32], f16, tag="cc2")
                rm = pool.tile([128, 66, 32], f16, tag="cand2")
                cm1 = pool.tile([128, 64, 32], f16, tag="cc2")
                cm = pool.tile([128, 64, 32], f16, tag="nmsCM")
                nc.vector.tensor_tensor(rm1[:], hp[:, :, 0:32], hp[:, :, 2:34], Alu.max)
                nc.vector.tensor_tensor(rm[:], rm1[:], hp[:, :, 1:33], Alu.max)
                nc.vector.tensor_tensor(cm1[:], rm[:, 0:64, :], rm[:, 2:66, :], Alu.max)
                nc.vector.tensor_tensor(cm[:], cm1[:], rm[:, 1:65, :], Alu.max)

                outt = iop.tile([128, 64, 32], f16, tag="out")
                nc.vector.tensor_tensor(outt[:], lm[:], cm[:], Alu.mult)
                eng = nc.scalar if r0 == 0 else nc.sync
                for h in (0, 1):
                    dst = bass.AP(out, (r0 + 64 * h) * W, [[32, 64], [W, 64], [1, 32]])
                    eng.dma_start(out=dst, in_=outt[64 * h:64 * h + 64, :, :])

            # interleave: per-channel flat chains with chunk-0 sections
            p0 = blocked_planes()
            for c in range(3):
                emit_flat_ch(c, chains[0:2])
                if c > 0:
                    emit_flat_ch(c - 1, chains[2:3])
                emit_blocked_ch(p0, CHUNKS[0][0], c)
            emit_flat_ch(2, chains[2:3])
            vbt1 = emit_vbt_load(CHUNKS[1][0], 0)
            vbt2 = emit_vbt_load(CHUNKS[1][0], 1)
            emit_blocked_nms(p0, CHUNKS[0][0])
            p1 = blocked_planes()
            emit_blocked_ch(p1, CHUNKS[1][0], 0, vbt=vbt1)
            emit_blocked_ch(p1, CHUNKS[1][0], 1, vbt=vbt2)
            emit_blocked_ch(p1, CHUNKS[1][0], 2)
            emit_blocked_nms(p1, CHUNKS[1][0])

    nc.finalize()
    return nc


def _get_compiled(low, high):
    key = (low, high)
    if key not in _COMPILED:
        _COMPILED[key] = _build(low, high)
    return _COMPILED[key]


def kernel(img, threshold1, threshold2, _trace=False):
    from concourse import bass_utils

    t1 = float(np.asarray(threshold1))
    t2 = float(np.asarray(threshold2))
    low, high = min(t1, t2), max(t1, t2)

    x = np.ascontiguousarray(np.asarray(img, dtype=np.float32)[0])  # [3,H,W]
    # pad columns with HALO zeros on both sides
    xp = np.zeros((3, H + 2 * HALO, PADW), dtype=np.float32)
    xp[:, HALO:HALO + H, HALO:HALO + W] = x
    wg = _wg_host()

    in_maps = []
    for k in range(8):
        shard = np.ascontiguousarray(xp[:, k * ROWS_PER_CORE:k * ROWS_PER_CORE + SHARD_ROWS, :])
        in_maps.append({"x": shard, "wg": wg})

    nc = _get_compiled(low, high)
    res = bass_utils.run_bass_kernel_spmd(nc, in_maps, core_ids=list(range(8)),
                                          trace=_trace)

    full = np.zeros((1, 1, H, W), dtype=np.float32)
    for k in range(8):
        full[0, 0, k * ROWS_PER_CORE:(k + 1) * ROWS_PER_CORE, :] = (
            res.results[k]["out"].astype(np.float32))
    # reference forces image borders to zero
    full[:, :, 0, :] = 0.0
    full[:, :, -1, :] = 0.0
    full[:, :, :, 0] = 0.0
    full[:, :, :, -1] = 0.0
    if _trace:
        kernel._last_results = res
    return full


# revision 26
# speedup vs baseline: 1.0038x; 1.0038x over previous
"""Canny edge detector on 8 Trainium2 NeuronCores.

Strategy (v2):
 - Stage F (flat): per channel, the vertical 5-tap gaussian vb = G_v(img)
   runs on the TENSOR engine as a banded fp32 matmul (exact to ~2e-7),
   in a flat rows-on-partitions layout (3 row-chains per core), written
   to an internal DRAM scratch `vbd`.
 - Stage B (blocked): identical to the v1 kernel but starts from vb
   (convs commute: G_h then A3_v/D3_v etc.), i.e. the 4 vertical-gaussian
   DVE ops per channel are gone.  Columns-on-partitions layout: partition
   p owns output cols [16p,16p+16) with a 26-wide halo window, all
   stencils are free-dim AP offsets.
 - Math (fp32, faithful to the reference):
     vb  = 5-tap vertical gaussian on img          (PE, stage F)
     bh  = 5-tap horizontal gaussian on vb         (DVE)
     t1  = vertical [1,2,1] of bh;  t2 = vertical [1,0,-1] of bh
     gx  = horizontal [1,0,-1] of t1;  gy = horizontal [1,2,1] of t2
     m_c = sqrt(gx^2+gy^2); g = sum_c m_c; sgx = sum_c gx; sgy = sum_c gy
     axis classification via |sgy| vs tan(22.5/67.5)*|sgx| and sign(sgx*sgy)
     nms+thresholds fused: cc = max of the 2 neighbors along the axis;
       hp = g > max(cc, high);  lm = g > max(cc, nextbefore(low))
     hysteresis: out = lm & max3x3(hp)   (binary planes in fp16)
"""

import numpy as np

_COMPILED = {}

H = 2048
W = 2048
HALO = 5
ROWS_PER_CORE = H // 8            # 256
SHARD_ROWS = ROWS_PER_CORE + 2 * HALO   # 266
PADW = W + 2 * HALO               # 2058
VB_ROWS = ROWS_PER_CORE + 6       # 262 (vb needs +-3 rows)
N_CHUNK = 128                     # output rows per chunk
CHUNKS = [(r, r + N_CHUNK) for r in range(0, ROWS_PER_CORE, N_CHUNK)]

# flat-stage chains: (img slab row start, img rows, out valid local range,
#                     vbd row start)   vb slab row v+2 == vbd row v
# chain A: img [0,128)   -> vb slab [2,126)   -> vbd [0,124)
# chain B: img [122,250) -> vb slab [124,248) -> vbd [122,244), write [124,244)
# chain C: img [244,266) -> vb slab [246,264) -> vbd [244,262)
SEGS = [(0, 512), (512, 512), (1024, 512), (1536, 512), (2048, 10)]


def _gauss5():
    return np.exp(-0.5 * (np.arange(5) - 2.0) ** 2).astype(np.float32)


def _wg_host():
    g5 = _gauss5()
    Wg = np.zeros((128, 124), np.float32)
    for i in range(124):
        for k in range(5):
            Wg[i + k, i] = g5[k]
    return Wg


def _build(low, high):
    import concourse.bass as bass
    import concourse.bacc as bacc
    import concourse.mybir as mybir
    from concourse.tile import TileContext

    f32 = mybir.dt.float32
    Alu = mybir.AluOpType
    Act = mybir.ActivationFunctionType

    g5 = _gauss5()
    ga = float(g5[0])
    gb = float(g5[1])
    t1c = float(np.float32(np.tan(np.deg2rad(np.float64(22.5)))))
    t2c = float(np.float32(np.tan(np.deg2rad(np.float64(67.5)))))

    nc = bacc.Bacc()
    x = nc.dram_tensor("x", [3, SHARD_ROWS, PADW], f32, kind="ExternalInput")
    wg = nc.dram_tensor("wg", [128, 124], f32, kind="ExternalInput")
    vbd = nc.dram_tensor("vbd", [3, VB_ROWS, PADW], f32, kind="Internal")
    f16o = mybir.dt.float16
    out = nc.dram_tensor("out", [ROWS_PER_CORE, W], f16o, kind="ExternalOutput")

    with TileContext(nc) as tc:
        with tc.tile_pool(name="io", bufs=2) as iop, tc.tile_pool(
            name="pl", bufs=1
        ) as pool, tc.tile_pool(name="fl", bufs=2) as flp, tc.tile_pool(
            name="ps", bufs=4, space="PSUM"
        ) as psp:
            # ---- Stage F: vb = G_v(img) on the PE, per channel/chain ----
            wgs = pool.tile([128, 124], f32, tag="wg")
            nc.sync.dma_start(out=wgs[:], in_=wg[:, :])

            # warm up the PE HAM clock gate with a dense dummy burst
            wps = psp.tile([124, 124], f32, tag="warm")
            for _ in range(10):
                nc.tensor.matmul(wps[:], lhsT=wgs[0:128, 0:124],
                                 rhs=wgs[0:128, 0:124], start=True, stop=True)

            # (img_row0, n_img_rows, lhsT cols (out rows), vb local base,
            #  vbd row0, n out rows written, local out row offset)
            chains = [
                (0, 128, 124, 0, 124, 0),
                (122, 128, 122, 124, 120, 2),
                (244, 22, 18, 244, 18, 0),
            ]

            def emit_flat_ch(c, chain_sel):
                for (i0, nr, m, v0, nw, oo) in chain_sel:
                    imgf = flp.tile([nr, PADW], f32, tag="imgf")
                    src = bass.AP(
                        x, c * SHARD_ROWS * PADW + i0 * PADW,
                        [[PADW, nr], [1, PADW]],
                    )
                    nc.sync.dma_start(out=imgf[:], in_=src)
                    vbf = flp.tile([m, PADW], f32, tag="vbf")
                    for (s0, sl) in SEGS:
                        ps = psp.tile([m, sl], f32, tag="ps")
                        nc.tensor.matmul(
                            ps[:], lhsT=wgs[0:nr, 0:m],
                            rhs=imgf[:, s0:s0 + sl],
                            start=True, stop=True,
                        )
                        nc.scalar.copy(vbf[:, s0:s0 + sl], ps[:])
                    dst = bass.AP(
                        vbd, c * VB_ROWS * PADW + v0 * PADW,
                        [[PADW, nw], [1, PADW]],
                    )
                    nc.sync.dma_start(out=dst, in_=vbf[oo:oo + nw, :])

            # ---- Stage B: blocked NMS pipeline from vbd ----
            # partition p = (h, b): row-half h (64 rows) x col-block b
            # (32 cols, 42-wide halo window)
            def blocked_planes():
                gpl = pool.tile([128, 68, 36], f32, tag="g")
                sgx = pool.tile([128, 68, 36], f32, tag="sgx")
                sgy = pool.tile([128, 68, 36], f32, tag="sgy")
                return gpl, sgx, sgy

            def emit_vbt_load(r0, c):
                vbt = iop.tile([128, 70, 42], f32, tag="img")
                for h in (0, 1):
                    src = bass.AP(
                        vbd, c * VB_ROWS * PADW + (r0 + 64 * h) * PADW,
                        [[32, 64], [PADW, 70], [1, 42]],
                    )
                    nc.sync.dma_start(out=vbt[64 * h:64 * h + 64, :, :], in_=src)
                return vbt

            def emit_blocked_ch(planes, r0, c, vbt=None):
                gpl, sgx, sgy = planes
                if True:
                    if vbt is None:
                        vbt = emit_vbt_load(r0, c)

                    s1 = pool.tile([128, 70, 38], f32, tag="tA")
                    s2 = pool.tile([128, 70, 38], f32, tag="tB")
                    bh1 = pool.tile([128, 70, 38], f32, tag="tC")
                    bh = pool.tile([128, 70, 38], f32, tag="tD")
                    # horizontal 5-tap gaussian [ga, gb, 1, gb, ga]
                    nc.vector.tensor_tensor(s1[:], vbt[:, :, 1:39], vbt[:, :, 3:41], Alu.add)
                    nc.vector.tensor_tensor(s2[:], vbt[:, :, 0:38], vbt[:, :, 4:42], Alu.add)
                    nc.vector.scalar_tensor_tensor(
                        bh1[:], s1[:], gb, vbt[:, :, 2:40], Alu.mult, Alu.add)
                    nc.vector.scalar_tensor_tensor(
                        bh[:], s2[:], ga, bh1[:], Alu.mult, Alu.add)

                    u = pool.tile([128, 68, 38], f32, tag="tA")
                    t1 = pool.tile([128, 68, 38], f32, tag="tB")
                    t2 = pool.tile([128, 68, 38], f32, tag="tC")
                    # vertical sobel components
                    nc.vector.tensor_tensor(u[:], bh[:, 0:68, :], bh[:, 2:70, :], Alu.add)
                    nc.vector.scalar_tensor_tensor(
                        t1[:], bh[:, 1:69, :], 2.0, u[:], Alu.mult, Alu.add)
                    nc.vector.tensor_tensor(t2[:], bh[:, 0:68, :], bh[:, 2:70, :], Alu.subtract)

                    gx = sgx if c == 0 else pool.tile([128, 68, 36], f32, tag="tD")
                    gy = sgy if c == 0 else pool.tile([128, 68, 36], f32, tag="tE")
                    w2 = pool.tile([128, 68, 36], f32, tag="tF")
                    # horizontal sobel components
                    nc.vector.tensor_tensor(gx[:], t1[:, :, 0:36], t1[:, :, 2:38], Alu.subtract)
                    nc.vector.tensor_tensor(w2[:], t2[:, :, 0:36], t2[:, :, 2:38], Alu.add)
                    nc.vector.scalar_tensor_tensor(
                        gy[:], t2[:, :, 1:37], 2.0, w2[:], Alu.mult, Alu.add)

                    q1 = pool.tile([128, 68, 36], f32, tag="tA")
                    q2 = pool.tile([128, 68, 36], f32, tag="tB")
                    r2 = pool.tile([128, 68, 36], f32, tag="tC")
                    m = gpl if c == 0 else pool.tile([128, 68, 36], f32, tag="tF")
                    nc.scalar.activation(q1[:], gx[:], Act.Square)
                    nc.scalar.activation(q2[:], gy[:], Act.Square)
                    nc.vector.tensor_tensor(r2[:], q1[:], q2[:], Alu.add)
                    nc.scalar.activation(m[:], r2[:], Act.Sqrt)

                    if c > 0:
                        nc.vector.tensor_tensor(gpl[:], gpl[:], m[:], Alu.add)
                        nc.vector.tensor_tensor(sgx[:], sgx[:], gx[:], Alu.add)
                        nc.vector.tensor_tensor(sgy[:], sgy[:], gy[:], Alu.add)

            def emit_blocked_nms(planes, r0):
                gpl, sgx, sgy = planes
                # ---- NMS ----
                u8 = mybir.dt.uint8
                rr = pool.tile([128, 66, 34], f32, tag="cand")
                ss = pool.tile([128, 66, 34], f32, tag="cand2")
                m0 = pool.tile([128, 66, 34], u8, tag="mk0")
                m2 = pool.tile([128, 66, 34], u8, tag="mk1")
                d = pool.tile([128, 66, 34], f32, tag="tE")
                dpos = pool.tile([128, 66, 34], u8, tag="mk2")
                nc.scalar.activation(rr[:], sgy[:, 1:67, 1:35], Act.Abs)
                nc.scalar.activation(ss[:], sgx[:, 1:67, 1:35], Act.Abs)
                nc.vector.scalar_tensor_tensor(m0[:], ss[:], t1c, rr[:], Alu.mult, Alu.is_ge)
                nc.vector.scalar_tensor_tensor(m2[:], ss[:], t2c, rr[:], Alu.mult, Alu.is_le)
                nc.vector.tensor_tensor(
                    d[:], sgx[:, 1:67, 1:35], sgy[:, 1:67, 1:35], Alu.mult)
                nc.vector.tensor_scalar(dpos[:], d[:], 0.0, None, Alu.is_ge)

                cand = pool.tile([128, 66, 34], f32, tag="cand")
                cc = pool.tile([128, 66, 34], f32, tag="cc")
                # base: c3 = max(SW, NE); overwrite with c1/c2/c0 by priority
                nc.vector.tensor_tensor(
                    cand[:], gpl[:, 2:68, 2:36], gpl[:, 0:66, 0:34], Alu.max)  # c1 SE/NW
                nc.vector.tensor_tensor(
                    cc[:], gpl[:, 2:68, 0:34], gpl[:, 0:66, 2:36], Alu.max)    # c3 SW/NE
                nc.vector.copy_predicated(cc[:], dpos[:], cand[:])
                cand2 = pool.tile([128, 66, 34], f32, tag="cand2")
                nc.vector.tensor_tensor(
                    cand2[:], gpl[:, 2:68, 1:35], gpl[:, 0:66, 1:35], Alu.max)  # c2 S/N
                nc.vector.copy_predicated(cc[:], m2[:], cand2[:])
                f16 = mybir.dt.float16
                cand3 = pool.tile([128, 66, 34], f32, tag="cand")
                nc.vector.tensor_tensor(
                    cand3[:], gpl[:, 1:67, 2:36], gpl[:, 1:67, 0:34], Alu.max)  # c0 E/W
                nc.vector.copy_predicated(cc[:], m0[:], cand3[:])

                hp = pool.tile([128, 66, 34], f16, tag="tF")
                lm = pool.tile([128, 64, 32], f16, tag="cand")
                lowx = float(np.nextafter(np.float32(low), np.float32(0.0)))
                nc.vector.scalar_tensor_tensor(
                    hp[:], cc[:], high, gpl[:, 1:67, 1:35], Alu.max, Alu.is_lt)
                nc.vector.scalar_tensor_tensor(
                    lm[:], cc[:, 1:65, 1:33], lowx, gpl[:, 2:66, 2:34],
                    Alu.max, Alu.is_lt)

                rm1 = pool.tile([128, 66, 32], f16, tag="cc2")
                rm = pool.tile([128, 66, 32], f16, tag="cand2")
                cm1 = pool.tile([128, 64, 32], f16, tag="cc2")
                cm = pool.tile([128, 64, 32], f16, tag="nmsCM")
                nc.vector.tensor_tensor(rm1[:], hp[:, :, 0:32], hp[:, :, 2:34], Alu.max)
                nc.vector.tensor_tensor(rm[:], rm1[:], hp[:, :, 1:33], Alu.max)
                nc.vector.tensor_tensor(cm1[:], rm[:, 0:64, :], rm[:, 2:66, :], Alu.max)
                nc.vector.tensor_tensor(cm[:], cm1[:], rm[:, 1:65, :], Alu.max)

                outt = iop.tile([128, 64, 32], f16, tag="out")
                nc.vector.tensor_tensor(outt[:], lm[:], cm[:], Alu.mult)
                for h in (0, 1):
                    dst = bass.AP(out, (r0 + 64 * h) * W, [[32, 64], [W, 64], [1, 32]])
                    nc.sync.dma_start(out=dst, in_=outt[64 * h:64 * h + 64, :, :])

            # interleave: per-channel flat chains with chunk-0 sections
            p0 = blocked_planes()
            for c in range(3):
                emit_flat_ch(c, chains[0:2])
                if c > 0:
                    emit_flat_ch(c - 1, chains[2:3])
                emit_blocked_ch(p0, CHUNKS[0][0], c)
            emit_flat_ch(2, chains[2:3])
            vbt1 = emit_vbt_load(CHUNKS[1][0], 0)
            emit_blocked_nms(p0, CHUNKS[0][0])
            p1 = blocked_planes()
            emit_blocked_ch(p1, CHUNKS[1][0], 0, vbt=vbt1)
            for c in range(1, 3):
                emit_blocked_ch(p1, CHUNKS[1][0], c)
            emit_blocked_nms(p1, CHUNKS[1][0])

    nc.finalize()
    return nc


def _get_compiled(low, high):
    key = (low, high)
    if key not in _COMPILED:
        _COMPILED[key] = _build(low, high)
    return _COMPILED[key]


def kernel(img, threshold1, threshold2, _trace=False):
    from concourse import bass_utils

    t1 = float(np.asarray(threshold1))
    t2 = float(np.asarray(threshold2))
    low, high = min(t1, t2), max(t1, t2)

    x = np.ascontiguousarray(np.asarray(img, dtype=np.float32)[0])  # [3,H,W]
    # pad columns with HALO zeros on both sides
    xp = np.zeros((3, H + 2 * HALO, PADW), dtype=np.float32)
    xp[:, HALO:HALO + H, HALO:HALO + W] = x
    wg = _wg_host()

    in_maps = []
    for k in range(8):
        shard = np.ascontiguousarray(xp[:, k * ROWS_PER_CORE:k * ROWS_PER_CORE + SHARD_ROWS, :])
        in_maps.append({"x": shard, "wg": wg})

    nc = _get_compiled(low, high)
    res = bass_utils.run_bass_kernel_spmd(nc, in_maps, core_ids=list(range(8)),
                                          trace=_trace)

    full = np.zeros((1, 1, H, W), dtype=np.float32)
    for k in range(8):
        full[0, 0, k * ROWS_PER_CORE:(k + 1) * ROWS_PER_CORE, :] = (
            res.results[k]["out"].astype(np.float32))
    # reference forces image borders to zero
    full[:, :, 0, :] = 0.0
    full[:, :, -1, :] = 0.0
    full[:, :, :, 0] = 0.0
    full[:, :, :, -1] = 0.0
    if _trace:
        kernel._last_results = res
    return full


# revision 28
# speedup vs baseline: 1.0044x; 1.0007x over previous
"""Canny edge detector on 8 Trainium2 NeuronCores.

Strategy (v2):
 - Stage F (flat): per channel, the vertical 5-tap gaussian vb = G_v(img)
   runs on the TENSOR engine as a banded fp32 matmul (exact to ~2e-7),
   in a flat rows-on-partitions layout (3 row-chains per core), written
   to an internal DRAM scratch `vbd`.
 - Stage B (blocked): identical to the v1 kernel but starts from vb
   (convs commute: G_h then A3_v/D3_v etc.), i.e. the 4 vertical-gaussian
   DVE ops per channel are gone.  Columns-on-partitions layout: partition
   p owns output cols [16p,16p+16) with a 26-wide halo window, all
   stencils are free-dim AP offsets.
 - Math (fp32, faithful to the reference):
     vb  = 5-tap vertical gaussian on img          (PE, stage F)
     bh  = 5-tap horizontal gaussian on vb         (DVE)
     t1  = vertical [1,2,1] of bh;  t2 = vertical [1,0,-1] of bh
     gx  = horizontal [1,0,-1] of t1;  gy = horizontal [1,2,1] of t2
     m_c = sqrt(gx^2+gy^2); g = sum_c m_c; sgx = sum_c gx; sgy = sum_c gy
     axis classification via |sgy| vs tan(22.5/67.5)*|sgx| and sign(sgx*sgy)
     nms+thresholds fused: cc = max of the 2 neighbors along the axis;
       hp = g > max(cc, high);  lm = g > max(cc, nextbefore(low))
     hysteresis: out = lm & max3x3(hp)   (binary planes in fp16)
"""

import numpy as np

_COMPILED = {}

H = 2048
W = 2048
HALO = 5
ROWS_PER_CORE = H // 8            # 256
SHARD_ROWS = ROWS_PER_CORE + 2 * HALO   # 266
PADW = W + 2 * HALO               # 2058
VB_ROWS = ROWS_PER_CORE + 6       # 262 (vb needs +-3 rows)
N_CHUNK = 128                     # output rows per chunk
CHUNKS = [(r, r + N_CHUNK) for r in range(0, ROWS_PER_CORE, N_CHUNK)]

# flat-stage chains: (img slab row start, img rows, out valid local range,
#                     vbd row start)   vb slab row v+2 == vbd row v
# chain A: img [0,128)   -> vb slab [2,126)   -> vbd [0,124)
# chain B: img [122,250) -> vb slab [124,248) -> vbd [122,244), write [124,244)
# chain C: img [244,266) -> vb slab [246,264) -> vbd [244,262)
SEGS = [(0, 512), (512, 512), (1024, 512), (1536, 512), (2048, 10)]


def _gauss5():
    return np.exp(-0.5 * (np.arange(5) - 2.0) ** 2).astype(np.float32)


def _wg_host():
    g5 = _gauss5()
    Wg = np.zeros((128, 124), np.float32)
    for i in range(124):
        for k in range(5):
            Wg[i + k, i] = g5[k]
    return Wg


def _build(low, high):
    import concourse.bass as bass
    import concourse.bacc as bacc
    import concourse.mybir as mybir
    from concourse.tile import TileContext

    f32 = mybir.dt.float32
    Alu = mybir.AluOpType
    Act = mybir.ActivationFunctionType

    g5 = _gauss5()
    ga = float(g5[0])
    gb = float(g5[1])
    t1c = float(np.float32(np.tan(np.deg2rad(np.float64(22.5)))))
    t2c = float(np.float32(np.tan(np.deg2rad(np.float64(67.5)))))

    nc = bacc.Bacc()
    x = nc.dram_tensor("x", [3, SHARD_ROWS, PADW], f32, kind="ExternalInput")
    wg = nc.dram_tensor("wg", [128, 124], f32, kind="ExternalInput")
    vbd = nc.dram_tensor("vbd", [3, VB_ROWS, PADW], f32, kind="Internal")
    f16o = mybir.dt.float16
    out = nc.dram_tensor("out", [ROWS_PER_CORE, W], f16o, kind="ExternalOutput")

    with TileContext(nc) as tc:
        with tc.tile_pool(name="io", bufs=2) as iop, tc.tile_pool(
            name="pl", bufs=1
        ) as pool, tc.tile_pool(name="fl", bufs=3) as flp, tc.tile_pool(
            name="ps", bufs=4, space="PSUM"
        ) as psp:
            # ---- Stage F: vb = G_v(img) on the PE, per channel/chain ----
            wgs = pool.tile([128, 124], f32, tag="wg")
            nc.sync.dma_start(out=wgs[:], in_=wg[:, :])

            # warm up the PE HAM clock gate with a dense dummy burst
            wps = psp.tile([124, 124], f32, tag="warm")
            for _ in range(10):
                nc.tensor.matmul(wps[:], lhsT=wgs[0:128, 0:124],
                                 rhs=wgs[0:128, 0:124], start=True, stop=True)

            # (img_row0, n_img_rows, lhsT cols (out rows), vb local base,
            #  vbd row0, n out rows written, local out row offset)
            chains = [
                (0, 128, 124, 0, 124, 0),
                (122, 128, 122, 124, 120, 2),
                (244, 22, 18, 244, 18, 0),
            ]

            def emit_flat_ch(c, chain_sel):
                for (i0, nr, m, v0, nw, oo) in chain_sel:
                    imgf = flp.tile([nr, PADW], f32, tag="imgf")
                    src = bass.AP(
                        x, c * SHARD_ROWS * PADW + i0 * PADW,
                        [[PADW, nr], [1, PADW]],
                    )
                    nc.sync.dma_start(out=imgf[:], in_=src)
                    vbf = flp.tile([m, PADW], f32, tag="vbf")
                    for (s0, sl) in SEGS:
                        ps = psp.tile([m, sl], f32, tag="ps")
                        nc.tensor.matmul(
                            ps[:], lhsT=wgs[0:nr, 0:m],
                            rhs=imgf[:, s0:s0 + sl],
                            start=True, stop=True,
                        )
                        nc.scalar.copy(vbf[:, s0:s0 + sl], ps[:])
                    dst = bass.AP(
                        vbd, c * VB_ROWS * PADW + v0 * PADW,
                        [[PADW, nw], [1, PADW]],
                    )
                    nc.sync.dma_start(out=dst, in_=vbf[oo:oo + nw, :])

            # ---- Stage B: blocked NMS pipeline from vbd ----
            # partition p = (h, b): row-half h (64 rows) x col-block b
            # (32 cols, 42-wide halo window)
            def blocked_planes():
                gpl = pool.tile([128, 68, 36], f32, tag="g")
                sgx = pool.tile([128, 68, 36], f32, tag="sgx")
                sgy = pool.tile([128, 68, 36], f32, tag="sgy")
                return gpl, sgx, sgy

            def emit_vbt_load(r0, c):
                vbt = iop.tile([128, 70, 42], f32, tag="img")
                for h in (0, 1):
                    src = bass.AP(
                        vbd, c * VB_ROWS * PADW + (r0 + 64 * h) * PADW,
                        [[32, 64], [PADW, 70], [1, 42]],
                    )
                    nc.sync.dma_start(out=vbt[64 * h:64 * h + 64, :, :], in_=src)
                return vbt

            def emit_blocked_ch(planes, r0, c, vbt=None):
                gpl, sgx, sgy = planes
                if True:
                    if vbt is None:
                        vbt = emit_vbt_load(r0, c)

                    s1 = pool.tile([128, 70, 38], f32, tag="tA")
                    s2 = pool.tile([128, 70, 38], f32, tag="tB")
                    bh1 = pool.tile([128, 70, 38], f32, tag="tC")
                    bh = pool.tile([128, 70, 38], f32, tag="tD")
                    # horizontal 5-tap gaussian [ga, gb, 1, gb, ga]
                    nc.vector.tensor_tensor(s1[:], vbt[:, :, 1:39], vbt[:, :, 3:41], Alu.add)
                    nc.vector.tensor_tensor(s2[:], vbt[:, :, 0:38], vbt[:, :, 4:42], Alu.add)
                    nc.vector.scalar_tensor_tensor(
                        bh1[:], s1[:], gb, vbt[:, :, 2:40], Alu.mult, Alu.add)
                    nc.vector.scalar_tensor_tensor(
                        bh[:], s2[:], ga, bh1[:], Alu.mult, Alu.add)

                    u = pool.tile([128, 68, 38], f32, tag="tA")
                    t1 = pool.tile([128, 68, 38], f32, tag="tB")
                    t2 = pool.tile([128, 68, 38], f32, tag="tC")
                    # vertical sobel components
                    nc.vector.tensor_tensor(u[:], bh[:, 0:68, :], bh[:, 2:70, :], Alu.add)
                    nc.vector.scalar_tensor_tensor(
                        t1[:], bh[:, 1:69, :], 2.0, u[:], Alu.mult, Alu.add)
                    nc.vector.tensor_tensor(t2[:], bh[:, 0:68, :], bh[:, 2:70, :], Alu.subtract)

                    gx = sgx if c == 0 else pool.tile([128, 68, 36], f32, tag="tD")
                    gy = sgy if c == 0 else pool.tile([128, 68, 36], f32, tag="tE")
                    w2 = pool.tile([128, 68, 36], f32, tag="tF")
                    # horizontal sobel components
                    nc.vector.tensor_tensor(gx[:], t1[:, :, 0:36], t1[:, :, 2:38], Alu.subtract)
                    nc.vector.tensor_tensor(w2[:], t2[:, :, 0:36], t2[:, :, 2:38], Alu.add)
                    nc.vector.scalar_tensor_tensor(
                        gy[:], t2[:, :, 1:37], 2.0, w2[:], Alu.mult, Alu.add)

                    q1 = pool.tile([128, 68, 36], f32, tag="tA")
                    q2 = pool.tile([128, 68, 36], f32, tag="tB")
                    r2 = pool.tile([128, 68, 36], f32, tag="tC")
                    m = gpl if c == 0 else pool.tile([128, 68, 36], f32, tag="tF")
                    nc.scalar.activation(q1[:], gx[:], Act.Square)
                    nc.scalar.activation(q2[:], gy[:], Act.Square)
                    nc.vector.tensor_tensor(r2[:], q1[:], q2[:], Alu.add)
                    nc.scalar.activation(m[:], r2[:], Act.Sqrt)

                    if c > 0:
                        nc.vector.tensor_tensor(gpl[:], gpl[:], m[:], Alu.add)
                        nc.vector.tensor_tensor(sgx[:], sgx[:], gx[:], Alu.add)
                        nc.vector.tensor_tensor(sgy[:], sgy[:], gy[:], Alu.add)

            def emit_blocked_nms(planes, r0):
                gpl, sgx, sgy = planes
                # ---- NMS ----
                u8 = mybir.dt.uint8
                rr = pool.tile([128, 66, 34], f32, tag="cand")
                ss = pool.tile([128, 66, 34], f32, tag="cand2")
                m0 = pool.tile([128, 66, 34], u8, tag="mk0")
                m2 = pool.tile([128, 66, 34], u8, tag="mk1")
                d = pool.tile([128, 66, 34], f32, tag="tE")
                dpos = pool.tile([128, 66, 34], u8, tag="mk2")
                nc.scalar.activation(rr[:], sgy[:, 1:67, 1:35], Act.Abs)
                nc.scalar.activation(ss[:], sgx[:, 1:67, 1:35], Act.Abs)
                nc.vector.scalar_tensor_tensor(m0[:], ss[:], t1c, rr[:], Alu.mult, Alu.is_ge)
                nc.vector.scalar_tensor_tensor(m2[:], ss[:], t2c, rr[:], Alu.mult, Alu.is_le)
                nc.vector.tensor_tensor(
                    d[:], sgx[:, 1:67, 1:35], sgy[:, 1:67, 1:35], Alu.mult)
                nc.vector.tensor_scalar(dpos[:], d[:], 0.0, None, Alu.is_ge)

                cand = pool.tile([128, 66, 34], f32, tag="cand")
                cc = pool.tile([128, 66, 34], f32, tag="cc")
                # base: c3 = max(SW, NE); overwrite with c1/c2/c0 by priority
                nc.vector.tensor_tensor(
                    cand[:], gpl[:, 2:68, 2:36], gpl[:, 0:66, 0:34], Alu.max)  # c1 SE/NW
                nc.vector.tensor_tensor(
                    cc[:], gpl[:, 2:68, 0:34], gpl[:, 0:66, 2:36], Alu.max)    # c3 SW/NE
                nc.vector.copy_predicated(cc[:], dpos[:], cand[:])
                cand2 = pool.tile([128, 66, 34], f32, tag="cand2")
                nc.vector.tensor_tensor(
                    cand2[:], gpl[:, 2:68, 1:35], gpl[:, 0:66, 1:35], Alu.max)  # c2 S/N
                nc.vector.copy_predicated(cc[:], m2[:], cand2[:])
                f16 = mybir.dt.float16
                cand3 = pool.tile([128, 66, 34], f32, tag="cand")
                nc.vector.tensor_tensor(
                    cand3[:], gpl[:, 1:67, 2:36], gpl[:, 1:67, 0:34], Alu.max)  # c0 E/W
                nc.vector.copy_predicated(cc[:], m0[:], cand3[:])

                hp = pool.tile([128, 66, 34], f16, tag="tF")
                lm = pool.tile([128, 64, 32], f16, tag="cand")
                lowx = float(np.nextafter(np.float32(low), np.float32(0.0)))
                nc.vector.scalar_tensor_tensor(
                    hp[:], cc[:], high, gpl[:, 1:67, 1:35], Alu.max, Alu.is_lt)
                nc.vector.scalar_tensor_tensor(
                    lm[:], cc[:, 1:65, 1:33], lowx, gpl[:, 2:66, 2:34],
                    Alu.max, Alu.is_lt)

                rm1 = pool.tile([128, 66, 32], f16, tag="cc2")
                rm = pool.tile([128, 66, 32], f16, tag="cand2")
                cm1 = pool.tile([128, 64, 32], f16, tag="cc2")
                cm = pool.tile([128, 64, 32], f16, tag="tE")
                nc.vector.tensor_tensor(rm1[:], hp[:, :, 0:32], hp[:, :, 2:34], Alu.max)
                nc.vector.tensor_tensor(rm[:], rm1[:], hp[:, :, 1:33], Alu.max)
                nc.vector.tensor_tensor(cm1[:], rm[:, 0:64, :], rm[:, 2:66, :], Alu.max)
                nc.vector.tensor_tensor(cm[:], cm1[:], rm[:, 1:65, :], Alu.max)

                outt = iop.tile([128, 64, 32], f16, tag="out")
                nc.vector.tensor_tensor(outt[:], lm[:], cm[:], Alu.mult)
                for h in (0, 1):
                    dst = bass.AP(out, (r0 + 64 * h) * W, [[32, 64], [W, 64], [1, 32]])
                    nc.sync.dma_start(out=dst, in_=outt[64 * h:64 * h + 64, :, :])

            # interleave: per-channel flat chains with chunk-0 sections
            p0 = blocked_planes()
            for c in range(3):
                emit_flat_ch(c, chains[0:2])
                if c > 0:
                    emit_flat_ch(c - 1, chains[2:3])
                emit_blocked_ch(p0, CHUNKS[0][0], c)
            emit_flat_ch(2, chains[2:3])
            vbt1 = emit_vbt_load(CHUNKS[1][0], 0)
            emit_blocked_nms(p0, CHUNKS[0][0])
            p1 = blocked_planes()
            emit_blocked_ch(p1, CHUNKS[1][0], 0, vbt=vbt1)
            for c in range(1, 3):
                emit_blocked_ch(p1, CHUNKS[1][0], c)
            emit_blocked_nms(p1, CHUNKS[1][0])

    nc.finalize()
    return nc


def _get_compiled(low, high):
    key = (low, high)
    if key not in _COMPILED:
        _COMPILED[key] = _build(low, high)
    return _COMPILED[key]


def kernel(img, threshold1, threshold2, _trace=False):
    from concourse import bass_utils

    t1 = float(np.asarray(threshold1))
    t2 = float(np.asarray(threshold2))
    low, high = min(t1, t2), max(t1, t2)

    x = np.ascontiguousarray(np.asarray(img, dtype=np.float32)[0])  # [3,H,W]
    # pad columns with HALO zeros on both sides
    xp = np.zeros((3, H + 2 * HALO, PADW), dtype=np.float32)
    xp[:, HALO:HALO + H, HALO:HALO + W] = x
    wg = _wg_host()

    in_maps = []
    for k in range(8):
        shard = np.ascontiguousarray(xp[:, k * ROWS_PER_CORE:k * ROWS_PER_CORE + SHARD_ROWS, :])
        in_maps.append({"x": shard, "wg": wg})

    nc = _get_compiled(low, high)
    res = bass_utils.run_bass_kernel_spmd(nc, in_maps, core_ids=list(range(8)),
                                          trace=_trace)

    full = np.zeros((1, 1, H, W), dtype=np.float32)
    for k in range(8):
        full[0, 0, k * ROWS_PER_CORE:(k + 1) * ROWS_PER_CORE, :] = (
            res.results[k]["out"].astype(np.float32))
    # reference forces image borders to zero
    full[:, :, 0, :] = 0.0
    full[:, :, -1, :] = 0.0
    full[:, :, :, 0] = 0.0
    full[:, :, :, -1] = 0.0
    if _trace:
        kernel._last_results = res
    return full
